# revision 37
# baseline (speedup 1.0000x reference)
"""Trainium2 Bass kernel for SAGAN-style self-attention (nn_Attention).

Reference computation (per batch b):
  f = Wf @ x + bf            [32, N]   (N = 64*64 = 4096 pixels)
  g = Wg @ y + bg            [32, N]
  h = Wh @ y + bh            [64, N]
  s[m, n] = sum_c g[c, m] f[c, n]
  beta = softmax(s, axis=n)
  o[m, c] = sum_n beta[m, n] h[c, n]
  out = gamma * o^T + x      [64, N]

Sharding: 8 cores = 4 batches x 2 query-halves. Each core computes the full
softmax rows for its 2048 queries (m) against all 4096 keys (n). The key axis
is permuted host-side so each core's own query half occupies columns 0:2048
-> identical SPMD program on all cores.

On-chip algorithm (St orientation: keys on partitions, m on free dim):
  St[n, m] = f[:, n].T @ g          (K=32 row-tiled bf16 matmuls, 4 bands)
  E = exp(St)  -- split across TWO engines:
     * ScalarE ACT Exp (exact), bf16 out
     * VectorE Schraudolph bit-trick: i32 = int(St*(2^23/ln2) + bias); the
       fp32 bit pattern of i32 is ~exp(St).  The O' matmul reads the top 16
       bits of each int32 directly as a stride-2 bf16 view (no extra copy).
       Softmax self-normalization cancels the approx error.
  O'[c|Z, m] = [hT | 1/gamma].T @ E  (K=128 accumulated over 32 key chunks;
       O' is emitted one pipeline step late and k-outer so the 4 m-bank
       matmuls of a chunk share one weight load and stream back-to-back)
  out[c, m] = O'[c, m] * recip(Z'[m]) + x[c, m]   (per-512 bank chains)
Softmax max-subtraction is skipped: |s| <= ~9 here, exp is safe in fp32.
"""
import numpy as np
import ml_dtypes

import bass_rust
import concourse.bass as bass

import concourse.mybir as mybir
import concourse.tile as tile
from concourse.bass_utils import run_bass_kernel_spmd


F32 = mybir.dt.float32
F32R = mybir.dt.float32r
I32 = mybir.dt.int32
BF16 = mybir.dt.bfloat16
AF = mybir.ActivationFunctionType
ALU = mybir.AluOpType

B, C, N = 4, 64, 4096
M = N // 2              # queries per core
CH = 64
MCH = 512               # m per matmul (one PSUM bank)

# Schraudolph exp-in-int-bits constants (bias tuned for this pipeline incl.
# bf16 truncation of the top half).
EXP_A = float(np.float32(2.0 ** 23 / np.log(2.0)))
EXP_B = float(np.float32((127 << 23) + 90000))

# which of the 8 exp tiles per quad go to the DVE (index = 2*mj + h)
DVE_TILES = {
    0: (3, 5),              # quad 0: DVE also does deferred projection casts
    7: (0, 1),              # quad 7: DVE also runs the output chains
}
DVE_TILES_DEFAULT = (1, 3, 5)


def split_multi_waits(nc, max_waits=1):
    """This walrus build supports a single sync-wait per instruction; spill
    extras onto fresh same-engine NOPs placed right before the instruction."""
    n_spill = 0
    for f in nc.m.functions:
        for bb in f.blocks:
            out = []
            changed = False
            for inst in bb.instructions:
                si = inst.sync_info
                if si is not None and len(si.on_wait) > max_waits:
                    waits = list(si.on_wait)
                    spill, keep = waits[:-max_waits], waits[-max_waits:]
                    for j in range(0, len(spill), max_waits):
                        n_spill += 1
                        out.append(
                            mybir.InstNoOp(
                                name=f"I-waitspill-{n_spill}",
                                engine=inst.engine,
                                bass_nofuse=True,
                                sync_info=mybir.SyncInfo(
                                    on_wait=spill[j : j + max_waits], on_update=[]
                                ),
                            )
                        )
                    inst.sync_info = bass_rust.SyncInfo(
                        on_update=list(si.on_update), on_wait=keep
                    )
                    changed = True
                out.append(inst)
            if changed:
                bb.instructions = out
    return n_spill


def build_kernel():
    nc = bass.Bass("TRN2", target_bir_lowering=False, debug=False, num_devices=8)

    # bf16 inputs are pre-augmented with a ones row (for the bias fold) and
    # pre-permuted so this core's queries are always columns 0:M.
    xab = nc.dram_tensor("xab", [C + 1, N], BF16, kind="ExternalInput").ap()
    yab = nc.dram_tensor("yab", [C + 1, N], BF16, kind="ExternalInput").ap()
    xres = nc.dram_tensor("xres", [C, M], F32, kind="ExternalInput").ap()
    wf4 = nc.dram_tensor("wf4", [C + 1, 128], BF16, kind="ExternalInput").ap()
    wg4 = nc.dram_tensor("wg4", [C + 1, 128], BF16, kind="ExternalInput").ap()
    wh = nc.dram_tensor("wh", [C + 1, CH], BF16, kind="ExternalInput").ap()
    # recip128 holds 1/gamma (folded into the Z column of hT)
    recip128 = nc.dram_tensor("recip128", [128, 32], F32, kind="ExternalInput").ap()
    out = nc.dram_tensor("out", [C, M], F32, kind="ExternalOutput").ap()

    with tile.TileContext(nc) as tc:
        with (
            tc.tile_pool(name="persist", bufs=1) as sb,
            tc.tile_pool(name="epool", bufs=14) as ep,
            tc.tile_pool(name="ipool", bufs=10) as ip,
            tc.tile_pool(name="scratch", bufs=2) as sc,
            tc.tile_pool(name="pst", bufs=2, space="PSUM") as pst,
            tc.tile_pool(name="pacc", bufs=1, space="PSUM") as pacc,
        ):
            # --- tiny dummy exp: trigger the ACT table load ASAP.  The
            # memset runs on GpSimd so the scalar queue's table load isn't
            # stuck behind input DMAs. ---
            dm = sc.tile([1, 1], F32, tag="dummy")
            nc.gpsimd.memset(dm[:], 0.0)
            dme = sc.tile([1, 1], F32, tag="dummy")
            nc.scalar.activation(dme[:], dm[:], AF.Exp)

            # --- input DMAs, spread across both HWDGE queues ---
            wf4_sb = sb.tile([C + 1, 128], BF16)
            wg4_sb = sb.tile([C + 1, 128], BF16)
            wh_sb = sb.tile([C + 1, CH], BF16)
            recip128_sb = sb.tile([128, 32], F32)
            y_m = sb.tile([C + 1, M], BF16)
            x_m = sb.tile([C + 1, M], BF16)
            y_h = sb.tile([C + 1, M], BF16)
            x_h = sb.tile([C + 1, M], BF16)
            xres_sb = sb.tile([C, M], F32)

            nc.sync.dma_start(wg4_sb[:], wg4[:])
            nc.sync.dma_start(wf4_sb[:], wf4[:])
            nc.sync.dma_start(y_m[:, bass.ts(0, 1024)], yab[:, bass.ts(0, 1024)])
            nc.sync.dma_start(x_m[:, bass.ts(0, 1024)], xab[:, bass.ts(0, 1024)])
            nc.sync.dma_start(y_m[:, bass.ts(1, 1024)], yab[:, bass.ts(1, 1024)])
            nc.sync.dma_start(x_m[:, bass.ts(1, 1024)], xab[:, bass.ts(1, 1024)])
            nc.sync.dma_start(wh_sb[:], wh[:])
            nc.sync.dma_start(recip128_sb[:], recip128[:])
            for j in range(2):
                nc.sync.dma_start(
                    y_h[:, bass.ts(j, 1024)], yab[:, bass.ds(M + 1024 * j, 1024)]
                )
            nc.sync.dma_start(xres_sb[:], xres[:])
            # qScalar: x_h (needed for the deferred f4 round in quad 0)
            for j in range(2):
                nc.scalar.dma_start(
                    x_h[:, bass.ts(j, 1024)], xab[:, bass.ds(M + 1024 * j, 1024)]
                )

            # --- projections (bf16 matmuls) ---
            # g4/f4 round 0 gate the first exp: per-512 tiles through the
            # double-buffered pst pool so MM(k+1) overlaps CAST(k), ordered
            # so St(q0, mj0) inputs (f4 keys 0:512, g4 bank 0) land first.
            # The first matmuls also warm the PE clock gate.
            g4_sb = sb.tile([128, M], BF16)
            f4_sb = sb.tile([128, N], BF16)

            def emit_proj_512(w_sb, src_t, src_off, dst_sb, dst_off):
                ps = pst.tile([128, MCH], F32, tag="st")
                nc.tensor.matmul(
                    ps[:], w_sb[:], src_t[:, bass.ds(src_off, MCH)],
                    start=True, stop=True,
                )
                nc.vector.tensor_copy(dst_sb[:, bass.ds(dst_off, MCH)], ps[:])

            emit_proj_512(wf4_sb, x_m, 0, f4_sb, 0)
            emit_proj_512(wg4_sb, y_m, 0, g4_sb, 0)
            for jj in range(1, 4):
                emit_proj_512(wg4_sb, y_m, MCH * jj, g4_sb, MCH * jj)
            for jj in range(1, 4):
                emit_proj_512(wf4_sb, x_m, MCH * jj, f4_sb, MCH * jj)

            # f4 round 1 (keys 2048:4096) is deferred into quad 0; it uses
            # the pacc region (the only pacc user before the O' accumulator).
            def emit_f4_round(j):
                psf = pacc.tile([128, 2048], F32, tag="acc")
                for jj in range(4):
                    nc.tensor.matmul(
                        psf[:, bass.ts(jj, MCH)], wf4_sb[:],
                        x_h[:, bass.ts(jj, MCH)], start=True, stop=True,
                    )
                nc.vector.tensor_copy(f4_sb[:, bass.ts(j, 2048)], psf[:])

            # hT_all: 32 chunks of [128, 65]; cols 65k..65k+64 = hT of key
            # chunk k (keys on partitions), col 65k+64 = 1/gamma (Z column).
            hT_all = sb.tile([128, 32 * (CH + 1)], BF16)

            def emit_hT_round(t):
                psh = pacc.tile([128, 2048], F32, tag="acc")
                for u in range(8):
                    k = 8 * t + u
                    ysrc = (
                        y_m[:, bass.ts(k, 128)]
                        if k < 16
                        else y_h[:, bass.ts(k - 16, 128)]
                    )
                    nc.tensor.matmul(
                        psh[:, bass.ds(64 * u, 64)], ysrc, wh_sb[:],
                        start=True, stop=True,
                    )
                dst = hT_all[:].rearrange("p (k e) -> p k e", k=32)[
                    :, 8 * t : 8 * t + 8, 0:64
                ]
                nc.vector.tensor_copy(
                    dst, psh[:, 0:512].rearrange("p (a b) -> p a b", a=8)
                )

            # --- main loop: St -> exp -> O' accumulate ---
            opref = {}

            def e_rhs(ent, rr):
                """rhs AP for O' from an exp tile entry: ('b'|'i', tile)."""
                kind, t = ent
                if kind == "b":
                    return t[:, bass.ts(rr, MCH)]
                v = t[:].bitcast(BF16).rearrange("p (n two) -> p two n", two=2)
                return v[:, bass.ds(1, 1), bass.ts(rr, MCH)]

            def emit_exp(st_tile, to_dve):
                if to_dve:
                    ei = ip.tile([128, 1024], I32, tag="ei")
                    nc.vector.tensor_scalar(
                        ei[:], st_tile[:], EXP_A, EXP_B, ALU.mult, ALU.add
                    )
                    return ("i", ei)
                e_t = ep.tile([128, 1024], BF16, tag="e")
                nc.scalar.activation(e_t[:], st_tile[:], AF.Exp)
                return ("b", e_t)

            def emit_oprime_group(qsrc, r2, elist):
                # one hT chunk's weights, all 4 m-banks (weight reuse)
                k = 4 * qsrc + r2
                lhsT = hT_all[:, bass.ds(65 * k, 65)]
                for mj in range(4):
                    nc.tensor.matmul(
                        opref["op"][:, bass.ts(mj, MCH)],
                        lhsT,
                        e_rhs(elist[2 * mj + r2 // 2], r2 % 2),
                        start=(k == 0), stop=(k == 31),
                    )

            def emit_oprime_bank(qsrc, mj, elist):
                # accumulate key chunks 4qsrc..4qsrc+3 into m bank mj
                for r2 in range(4):
                    k = 4 * qsrc + r2
                    nc.tensor.matmul(
                        opref["op"][:, bass.ts(mj, MCH)],
                        hT_all[:, bass.ds(65 * k, 65)],
                        e_rhs(elist[2 * mj + r2 // 2], r2 % 2),
                        start=(k == 0), stop=(k == 31),
                    )

            def emit_oprime_pair23(elist):
                # final chunks 28..31 for banks 2+3 together so each hT
                # chunk's weights load once for both banks
                for r2 in range(4):
                    k = 28 + r2
                    lhsT = hT_all[:, bass.ds(65 * k, 65)]
                    for mj in (2, 3):
                        nc.tensor.matmul(
                            opref["op"][:, bass.ts(mj, MCH)],
                            lhsT,
                            e_rhs(elist[2 * mj + r2 // 2], r2 % 2),
                            start=False, stop=(k == 31),
                        )

            # tail chains: one per 512-wide m bank, stage-split so the four
            # banks pipeline.  zpipe: Z row -> [128,4] reshape -> reciprocal
            # -> broadcast-DMA into a [64,512] SBUF tile (0-stride repeat on
            # the source AP; no matmul, no PSUM, no F32R).  finish: multiply
            # + residual + output DMA (out-DMAs batched last on qSync).
            def _t(nm, shape, dt, n=4):
                return [
                    sc.tile(shape, dt, tag=f"{nm}{i}", name=f"{nm}{i}")
                    for i in range(n)
                ]

            ones65f = sb.tile([1, CH + 1], F32)
            nc.vector.memset(ones65f[:], 1.0)
            ones65r = sb.tile([1, CH + 1], F32R)
            nc.vector.tensor_scalar_mul(ones65r[:], ones65f[:], 1.0)

            zc = _t("zc", [1, MCH], F32)
            z128 = _t("z128", [128, 4], F32)
            zr = _t("zr", [128, 4], F32)
            zrr = _t("zrr", [128, 4], F32R)
            r0r = _t("r0r", [1, MCH], F32R)
            rb = _t("rb", [CH, MCH], F32)
            o_sb = _t("osb", [CH, MCH], F32)
            out_dmas = []

            def emit_zpipe(i):
                op = opref["op"]
                cs = bass.ts(i, MCH)
                nc.scalar.copy(zc[i][:], op[CH : CH + 1, cs])
                nc.sync.dma_start(z128[i][:], zc[i][:])  # SBUF reshape
                nc.vector.reciprocal(zr[i][:], z128[i][:])
                nc.vector.tensor_scalar_mul(zrr[i][:], zr[i][:], 1.0)
                nc.sync.dma_start(r0r[i][:], zrr[i][:])  # reshape back

            def emit_rbphase(i):
                rb_ps = pst.tile([CH + 1, MCH], F32, tag="st", name=f"rbps{i}")
                nc.tensor.matmul(
                    rb_ps[:], ones65r[:], r0r[i][:], start=True, stop=True,
                )
                nc.scalar.copy(rb[i][:], rb_ps[0:CH, :])

            def emit_finish(i):
                op = opref["op"]
                cs = bass.ts(i, MCH)
                nc.vector.tensor_mul(o_sb[i][:], op[0:CH, cs], rb[i][:])
                if i < 3:
                    nc.gpsimd.tensor_add(
                        o_sb[i][:], o_sb[i][:], xres_sb[:, cs]
                    )
                else:
                    nc.vector.tensor_add(
                        o_sb[i][:], o_sb[i][:], xres_sb[:, cs]
                    )
                out_dmas.append(
                    lambda cs=cs, i=i: nc.sync.dma_start(
                        out[:, cs], o_sb[i][:]
                    )
                )

            deferred = [lambda: emit_f4_round(1)] + [
                (lambda t=t: emit_hT_round(t)) for t in range(4)
            ]
            pend = []  # one-pipeline-step-delayed tensor-side work
            pend_pre = []  # like pend, but emitted BEFORE the St matmuls so
            # the O'(7) banks aren't FIFO-blocked behind a psum-waiting St
            eprev = None
            for q in range(8):
                dve_set = DVE_TILES.get(q, DVE_TILES_DEFAULT)
                ecur = []
                for mj in range(4):
                    for fn in pend_pre:
                        fn()
                    pend_pre = []
                    # St: 4 chunk matmuls on the 4 32-row PE bands.
                    sts = []
                    for h in range(2):
                        st = pst.tile([128, 1024], F32, tag="st")
                        for rr in range(2):
                            r = 2 * h + rr
                            nc.tensor.matmul(
                                st[:, bass.ts(rr, MCH)],
                                f4_sb[
                                    bass.ds(32 * r, 32), bass.ts(4 * q + r, 128)
                                ],
                                g4_sb[bass.ds(32 * r, 32), bass.ts(mj, MCH)],
                                start=True, stop=True,
                                tile_position=(32 * r, 0),
                            )
                        sts.append(st)
                    for h in range(2):
                        ecur.append(emit_exp(sts[h], (2 * mj + h) in dve_set))
                    # emit the delayed tensor-side work so it queues BEHIND
                    # the St matmuls the next exps depend on
                    for fn in pend:
                        fn()
                    pend = []
                    if q == 0:
                        for _ in range(2):
                            if deferred:
                                deferred.pop(0)()
                        if mj == 3:
                            zdst = hT_all[:].rearrange(
                                "p (k e) -> p k e", k=32
                            )[:, :, 64:65]
                            nc.vector.tensor_copy(
                                zdst,
                                recip128_sb[:].rearrange(
                                    "p (a b) -> p a b", a=32
                                ),
                            )
                            op_tile = pacc.tile([CH + 1, M], F32, tag="acc")
                            opref["op"] = op_tile
                    elif q < 7:
                        el, qq, mm = eprev, q - 1, mj
                        pend.append(
                            lambda el=el, qq=qq, mm=mm: emit_oprime_group(
                                qq, mm, el
                            )
                        )
                    else:
                        # q7: drain all O'(6) groups by step (7,2) so the Z
                        # rows are complete before any zpipe reads them; the
                        # O'(7) banks + zpipes follow, placed ahead of the St
                        # matmuls so they run inside St's psum-wait window.
                        el6, el7 = eprev, ecur
                        if mj < 2:
                            for r2 in (2 * mj, 2 * mj + 1):
                                pend.append(
                                    lambda el=el6, r2=r2: emit_oprime_group(
                                        6, r2, el
                                    )
                                )
                        elif mj == 2:
                            for b in (0, 1):
                                pend_pre.append(
                                    lambda el=el7, b=b: emit_oprime_bank(
                                        7, b, el
                                    )
                                )
                                pend.append(lambda b=b: emit_zpipe(b))
                        else:
                            pend_pre.append(
                                lambda el=el7: emit_oprime_pair23(el)
                            )
                            pend.append(lambda: emit_zpipe(2))
                            pend.append(lambda: emit_zpipe(3))
                eprev = ecur
            for fn in pend_pre:
                fn()
            for fn in pend:
                fn()
            # tail: rb matmuls + copies, then normalize/residual per bank.
            # rbphase(i+2) reuses the pst buffer freed by rbphase(i)'s copy.
            emit_rbphase(0)
            emit_rbphase(1)
            emit_finish(0)
            emit_rbphase(2)
            emit_finish(1)
            emit_rbphase(3)
            emit_finish(2)
            emit_finish(3)
            for fn in out_dmas:
                fn()

    split_multi_waits(nc)
    return nc


def make_in_maps(x, y, Wf, bf, Wg, bg, Wh, bh, gamma):
    x = np.asarray(x, dtype=np.float32).reshape(B, C, N)
    y = np.asarray(y, dtype=np.float32).reshape(B, C, N)
    bf16 = ml_dtypes.bfloat16
    wf4 = np.tile(
        np.concatenate([np.asarray(Wf).T, np.asarray(bf)[None, :]], 0), (1, 4)
    ).astype(bf16)
    wg4 = np.tile(
        np.concatenate([np.asarray(Wg).T, np.asarray(bg)[None, :]], 0), (1, 4)
    ).astype(bf16)
    wh = np.concatenate(
        [np.asarray(Wh).T, np.asarray(bh)[None, :]], 0
    ).astype(bf16)
    inv_gamma = 1.0 / float(np.asarray(gamma).reshape(-1)[0])
    recip128 = np.full((128, 32), inv_gamma, np.float32)
    onesr = np.ones((1, N), np.float32)

    in_maps = []
    for core in range(8):
        b, half = core // 2, core % 2
        mine = slice(half * M, half * M + M)
        other = slice((1 - half) * M, (1 - half) * M + M)
        xa = np.concatenate([x[b][:, mine], x[b][:, other]], axis=1)
        ya = np.concatenate([y[b][:, mine], y[b][:, other]], axis=1)
        xab = np.concatenate([xa, onesr], axis=0).astype(bf16)
        yab = np.concatenate([ya, onesr], axis=0).astype(bf16)
        in_maps.append(
            {
                "xab": np.ascontiguousarray(xab),
                "yab": np.ascontiguousarray(yab),
                "xres": np.ascontiguousarray(x[b][:, mine]),
                "wf4": wf4, "wg4": wg4, "wh": wh,
                "recip128": recip128,
            }
        )
    return in_maps


def assemble_output(results):
    o = np.empty((B, C, N), np.float32)
    for core in range(8):
        b, half = core // 2, core % 2
        o[b][:, half * M : half * M + M] = results[core]["out"]
    return o.reshape(B, C, 64, 64)


_NC_CACHE = {}


def run(trace=False, **inputs):
    if "nc" not in _NC_CACHE:
        _NC_CACHE["nc"] = build_kernel()
    nc = _NC_CACHE["nc"]
    in_maps = make_in_maps(**inputs)
    res = run_bass_kernel_spmd(nc, in_maps, list(range(8)), trace=trace)
    return assemble_output(res.results), res


def kernel(**inputs):
    out, _ = run(trace=False, **inputs)
    return out
